# revision 1
# baseline (speedup 1.0000x reference)
"""MoE experts kernel for TRN2, expert-parallel over 8 NeuronCores.

Reference computation (T=4096, E=8, H=1024, Q=1024):
    gate_up = einsum('th,ehq->teq', x, gate_up_proj)      # (T, E, 2Q)
    gate, up = split(gate_up, 2, axis=-1)
    hidden = silu(gate) * up                              # (T, E, Q)
    expert_outputs = einsum('teq,eqh->teh', hidden, down_proj)
    out = einsum('teh,te->th', expert_outputs, routing_weights)

Sharding: expert-parallel. Core e computes its expert's full contribution
r[:, e] * (silu(x @ Wgu_gate) * (x @ Wgu_up)) @ Wdn  for all T tokens,
entirely in feature-major layout (features on partitions, tokens on the
free axis) so no on-device transposes are needed; the host sums the 8
partial outputs (the expert-parallel all-reduce) and transposes back.
"""

import sys

for _p in ("/opt/trn_rl_repo", "/root/.axon_site/_ro/trn_rl_repo"):
    if _p not in sys.path:
        sys.path.insert(0, _p)

import numpy as np

T, E, H, Q = 4096, 8, 1024, 1024
P = 128          # partitions
TC = 512         # token chunk (= one PSUM bank of fp32)
NT = T // TC     # 8 token chunks
KH = H // P      # 8 contraction tiles for the gate_up matmul
KQ = Q // P      # 8 contraction tiles for the down matmul
NH = H // P      # 8 output-feature tiles

_CACHED = None


def _split_waits(nc, max_waits=1):
    """Walrus codegen for several TRN2 ISA structs accepts only one sync-wait
    per instruction ("Too many sync wait commands"). Splitting is safe: a
    same-engine NoOp earlier in the (FIFO) stream carrying the extra waits
    blocks the stream at the same point the original multi-wait would have."""
    import concourse.mybir as mybir

    for f in nc.m.functions:
        for blk in f.blocks:
            newlist, changed = [], False
            for inst in blk.instructions:
                si = inst.sync_info
                if si is not None and si.on_wait and len(si.on_wait) > max_waits:
                    extra = si.on_wait[:-max_waits]
                    keep = si.on_wait[-max_waits:]
                    inst.sync_info = mybir.SyncInfo(
                        on_wait=list(keep), on_update=list(si.on_update or [])
                    )
                    for j, w in enumerate(extra):
                        nop = mybir.InstNoOp(
                            name=f"{inst.name}-wn{j}", engine=inst.engine
                        )
                        nop.sync_info = mybir.SyncInfo(on_wait=[w], on_update=[])
                        newlist.append(nop)
                    changed = True
                newlist.append(inst)
            if changed:
                blk.instructions = newlist


def _build():
    import concourse.bass as bass
    import concourse.mybir as mybir
    import concourse.tile as tile

    nc = bass.Bass("TRN2", target_bir_lowering=False, debug=False, num_devices=E)

    f32 = mybir.dt.float32
    # float32r: fp32 data matmul'd in "replicated" mode — 1 cycle/row for
    # moving dim >= 256 (same as bf16) with ~1e-4 relative error.
    f32r = mybir.dt.float32r

    xT_d = nc.dram_tensor("xT", [H, T], f32r, kind="ExternalInput").ap()
    # w_gu host-packed as (2Q/P slabs, P, KH, P): slab order is first-use
    # order (gate qi, up qi alternating); each slab is one contiguous DMA
    # with 4KB per partition.
    wgu_d = nc.dram_tensor(
        "w_gu", [2 * Q // P, P, KH, P], f32r, kind="ExternalInput"
    ).ap()
    wdn_d = nc.dram_tensor("w_dn", [Q, H], f32r, kind="ExternalInput").ap()
    rw_d = nc.dram_tensor("rw", [1, T], mybir.dt.float32, kind="ExternalInput").ap()
    out_d = nc.dram_tensor("out", [H, T], mybir.dt.float32, kind="ExternalOutput").ap()

    from contextlib import ExitStack

    with tile.TileContext(nc) as tc:
        with ExitStack() as es:
            consts = es.enter_context(tc.tile_pool(name="consts", bufs=1))
            psum_gu = es.enter_context(tc.tile_pool(name="psum_gu", bufs=2, space="PSUM"))
            psum_o = es.enter_context(tc.tile_pool(name="psum_o", bufs=4, space="PSUM"))
            xT_pool = es.enter_context(tc.tile_pool(name="xTc", bufs=2))
            hid_pool = es.enter_context(tc.tile_pool(name="hid", bufs=2))
            tmp_pool = es.enter_context(tc.tile_pool(name="tmp", bufs=2))
            r_pool = es.enter_context(tc.tile_pool(name="rch", bufs=2))
            ost_pool = es.enter_context(tc.tile_pool(name="ost", bufs=4))
            wgu_s = consts.tile([P, 2 * Q // P, KH, P], f32r)
            wdn_s = consts.tile([P, KQ, H], f32r)

            # Load order = first-use order: first W_gu slabs (host-packed in
            # gate/up alternating order), second-matmul weights, then the
            # rest; xT token chunks stream inside the loop (fp32 x doesn't
            # fit SBUF whole).
            # First gate/up slabs split into k-halves so the first matmuls
            # only wait for 512KB, not a full 1MB slab.
            nc.sync.dma_start(out=wgu_s[:, 0, 0:KH // 4], in_=wgu_d[0, :, 0:KH // 4])
            nc.sync.dma_start(out=wgu_s[:, 0, KH // 4:KH // 2], in_=wgu_d[0, :, KH // 4:KH // 2])
            xT_chunks = []
            xc0 = xT_pool.tile([P, KH, TC], f32r, tag="xc")
            for k in range(KH):
                nc.sync.dma_start(
                    out=xc0[:, k, :], in_=xT_d[k * P:(k + 1) * P, 0:TC]
                )
                if k == 0:
                    nc.sync.dma_start(
                        out=wgu_s[:, 0, KH // 2:], in_=wgu_d[0, :, KH // 2:]
                    )
                if k == 1:
                    nc.sync.dma_start(
                        out=wgu_s[:, 1, 0:KH // 2], in_=wgu_d[1, :, 0:KH // 2]
                    )
                if k == 2:
                    nc.sync.dma_start(
                        out=wgu_s[:, 1, KH // 2:], in_=wgu_d[1, :, KH // 2:]
                    )
            xT_chunks.append(xc0)
            for s in range(2, 2 * Q // P):
                nc.sync.dma_start(out=wgu_s[:, s], in_=wgu_d[s])
            xc1 = xT_pool.tile([P, KH, TC], f32r, tag="xc")
            for k in range(KH):
                nc.sync.dma_start(
                    out=xc1[:, k, :], in_=xT_d[k * P:(k + 1) * P, TC:2 * TC]
                )
            xT_chunks.append(xc1)
            for k in range(KQ):
                nc.sync.dma_start(out=wdn_s[:, k, :], in_=wdn_d[k * P:(k + 1) * P, :])

            for tci in range(NT):
                t0 = tci * TC
                xc = xT_chunks[tci]
                if tci + 2 < NT:
                    nxc = xT_pool.tile([P, KH, TC], f32r, tag="xc")
                    for k in range(KH):
                        nc.sync.dma_start(
                            out=nxc[:, k, :],
                            in_=xT_d[k * P:(k + 1) * P, (tci + 2) * TC:(tci + 3) * TC],
                        )
                    xT_chunks.append(nxc)
                r_c = r_pool.tile([P, TC], f32)
                nc.sync.dma_start(out=r_c, in_=rw_d[:, t0:t0 + TC].to_broadcast([P, TC]))
                hid = hid_pool.tile([P, KQ, TC], f32r)
                for qi in range(KQ):
                    gate_ps = psum_gu.tile([P, TC], f32, tag="gate")
                    up_ps = psum_gu.tile([P, TC], f32, tag="up")
                    for k in range(KH):
                        nc.tensor.matmul(
                            gate_ps,
                            wgu_s[:, 2 * qi, k, :],
                            xc[:, k, :],
                            start=(k == 0),
                            stop=(k == KH - 1),
                        )
                    for k in range(KH):
                        nc.tensor.matmul(
                            up_ps,
                            wgu_s[:, 2 * qi + 1, k, :],
                            xc[:, k, :],
                            start=(k == 0),
                            stop=(k == KH - 1),
                        )
                    tmp = tmp_pool.tile([P, TC], f32)
                    nc.scalar.activation(
                        tmp, gate_ps, mybir.ActivationFunctionType.Silu
                    )
                    nc.vector.tensor_mul(hid[:, qi, :], tmp, up_ps)

                for hi in range(NH):
                    o_ps = psum_o.tile([P, TC], f32)
                    for qi in range(KQ):
                        nc.tensor.matmul(
                            o_ps,
                            wdn_s[:, qi, hi * P:(hi + 1) * P],
                            hid[:, qi, :],
                            start=(qi == 0),
                            stop=(qi == KQ - 1),
                        )
                    ost = ost_pool.tile([P, TC], f32)
                    nc.vector.tensor_mul(ost, o_ps, r_c)
                    nc.sync.dma_start(
                        out=out_d[hi * P:(hi + 1) * P, t0:t0 + TC], in_=ost
                    )
    _split_waits(nc)
    return nc


def _get_nc():
    global _CACHED
    if _CACHED is None:
        _CACHED = _build()
    return _CACHED


def _pack_wgu(w):
    """(H, 2Q) -> (16, 128, KH, 128) slabs in first-use order: 128-column
    blocks interleaved gate qi / up qi, each slab partition-major."""
    w = np.asarray(w, dtype=np.float32)
    # (KH, P, n_blk, P): k-tile, partition, column block, column
    w4 = w.reshape(KH, P, 2 * Q // P, P)
    order = [b for qi in range(KQ) for b in (qi, KQ + qi)]
    # slab s: (P, KH, P)
    return np.ascontiguousarray(w4.transpose(2, 1, 0, 3)[order])


def kernel(x, routing_weights, gate_up_proj, down_proj):
    from concourse.bass_utils import run_bass_kernel_spmd

    nc = _get_nc()

    xT = np.ascontiguousarray(np.asarray(x, dtype=np.float32).T)
    rw = np.asarray(routing_weights, dtype=np.float32)
    in_maps = []
    for e in range(E):
        in_maps.append({
            "xT": xT,
            "w_gu": _pack_wgu(gate_up_proj[e]),
            "w_dn": np.ascontiguousarray(np.asarray(down_proj[e], dtype=np.float32)),
            "rw": np.ascontiguousarray(rw[:, e].reshape(1, T)),
        })

    res = run_bass_kernel_spmd(nc, in_maps, core_ids=list(range(E)))

    total = np.zeros((H, T), dtype=np.float32)
    for r in res.results:
        total += r["out"]
    return np.ascontiguousarray(total.T)



# revision 6
# speedup vs baseline: 1.0568x; 1.0568x over previous
"""MoE experts kernel for TRN2, expert-parallel over 8 NeuronCores.

Reference computation (T=4096, E=8, H=1024, Q=1024):
    gate_up = einsum('th,ehq->teq', x, gate_up_proj)      # (T, E, 2Q)
    gate, up = split(gate_up, 2, axis=-1)
    hidden = silu(gate) * up                              # (T, E, Q)
    expert_outputs = einsum('teq,eqh->teh', hidden, down_proj)
    out = einsum('teh,te->th', expert_outputs, routing_weights)

Sharding: expert-parallel. Core e computes its expert's full contribution
r[:, e] * (silu(x @ Wgu_gate) * (x @ Wgu_up)) @ Wdn  for all T tokens,
entirely in feature-major layout (features on partitions, tokens on the
free axis) so no on-device transposes are needed; the host sums the 8
partial outputs (the expert-parallel all-reduce) and transposes back.
"""

import sys

for _p in ("/opt/trn_rl_repo", "/root/.axon_site/_ro/trn_rl_repo"):
    if _p not in sys.path:
        sys.path.insert(0, _p)

import numpy as np

T, E, H, Q = 4096, 8, 1024, 1024
P = 128          # partitions
TC = 512         # token chunk (= one PSUM bank of fp32)
NT = T // TC     # 8 token chunks
KH = H // P      # 8 contraction tiles for the gate_up matmul
KQ = Q // P      # 8 contraction tiles for the down matmul
NH = H // P      # 8 output-feature tiles

_CACHED = None


def _split_waits(nc, max_waits=1):
    """Walrus codegen for several TRN2 ISA structs accepts only one sync-wait
    per instruction ("Too many sync wait commands"). Splitting is safe: a
    same-engine NoOp earlier in the (FIFO) stream carrying the extra waits
    blocks the stream at the same point the original multi-wait would have."""
    import concourse.mybir as mybir

    for f in nc.m.functions:
        for blk in f.blocks:
            newlist, changed = [], False
            for inst in blk.instructions:
                si = inst.sync_info
                if si is not None and si.on_wait and len(si.on_wait) > max_waits:
                    extra = si.on_wait[:-max_waits]
                    keep = si.on_wait[-max_waits:]
                    inst.sync_info = mybir.SyncInfo(
                        on_wait=list(keep), on_update=list(si.on_update or [])
                    )
                    for j, w in enumerate(extra):
                        nop = mybir.InstNoOp(
                            name=f"{inst.name}-wn{j}", engine=inst.engine
                        )
                        nop.sync_info = mybir.SyncInfo(on_wait=[w], on_update=[])
                        newlist.append(nop)
                    changed = True
                newlist.append(inst)
            if changed:
                blk.instructions = newlist


def _build():
    import concourse.bass as bass
    import concourse.mybir as mybir
    import concourse.tile as tile

    nc = bass.Bass("TRN2", target_bir_lowering=False, debug=False, num_devices=E)

    f32 = mybir.dt.float32
    # bf16: same PE rate as fp32r (1 cycle/row for moving >= 256) but half
    # the HBM traffic and half-width weight loads; quantization adds ~0.3%
    # relative error, well inside the 2e-2 gate.
    bf16 = mybir.dt.bfloat16

    xT_d = nc.dram_tensor("xT", [H, T], bf16, kind="ExternalInput").ap()
    # w_gu host-packed as (2Q/P slabs, P, KH, P): slab order is first-use
    # order (gate qi, up qi alternating); each slab is one contiguous DMA
    # with 2KB per partition.
    wgu_d = nc.dram_tensor(
        "w_gu", [2 * Q // P, P, KH, P], bf16, kind="ExternalInput"
    ).ap()
    wdn_d = nc.dram_tensor("w_dn", [Q, H], bf16, kind="ExternalInput").ap()
    rw_d = nc.dram_tensor("rw", [1, T], mybir.dt.float32, kind="ExternalInput").ap()
    out_d = nc.dram_tensor("out", [H, T], mybir.dt.float32, kind="ExternalOutput").ap()

    from contextlib import ExitStack

    with tile.TileContext(nc) as tc:
        with ExitStack() as es:
            consts = es.enter_context(tc.tile_pool(name="consts", bufs=1))
            psum_gu = es.enter_context(tc.tile_pool(name="psum_gu", bufs=2, space="PSUM"))
            psum_o = es.enter_context(tc.tile_pool(name="psum_o", bufs=4, space="PSUM"))
            xT_pool = es.enter_context(tc.tile_pool(name="xTc", bufs=2))
            hid_pool = es.enter_context(tc.tile_pool(name="hid", bufs=2))
            tmp_pool = es.enter_context(tc.tile_pool(name="tmp", bufs=2))
            r_pool = es.enter_context(tc.tile_pool(name="rch", bufs=2))
            ost_pool = es.enter_context(tc.tile_pool(name="ost", bufs=4))
            wgu_s = consts.tile([P, 2 * Q // P, KH, P], bf16)
            wdn_s = consts.tile([P, KQ, H], bf16)

            # Load order = first-use order: first W_gu slabs (host-packed in
            # gate/up alternating order), second-matmul weights, then the
            # rest; xT token chunks stream inside the loop (fp32 x doesn't
            # fit SBUF whole).
            # First gate/up slabs split into k-halves so the first matmuls
            # only wait for 512KB, not a full 1MB slab.
            nc.sync.dma_start(out=wgu_s[:, 0, 0:KH // 4], in_=wgu_d[0, :, 0:KH // 4])
            nc.sync.dma_start(out=wgu_s[:, 0, KH // 4:KH // 2], in_=wgu_d[0, :, KH // 4:KH // 2])
            xT_chunks = []
            xc0 = xT_pool.tile([P, KH, TC], bf16, tag="xc")
            for k in range(KH):
                nc.sync.dma_start(
                    out=xc0[:, k, :], in_=xT_d[k * P:(k + 1) * P, 0:TC]
                )
                if k == 0:
                    nc.sync.dma_start(
                        out=wgu_s[:, 0, KH // 2:], in_=wgu_d[0, :, KH // 2:]
                    )
                if k == 1:
                    nc.sync.dma_start(
                        out=wgu_s[:, 1, 0:KH // 2], in_=wgu_d[1, :, 0:KH // 2]
                    )
                if k == 2:
                    nc.sync.dma_start(
                        out=wgu_s[:, 1, KH // 2:], in_=wgu_d[1, :, KH // 2:]
                    )
            xT_chunks.append(xc0)
            for s in range(2, 2 * Q // P):
                nc.sync.dma_start(out=wgu_s[:, s], in_=wgu_d[s])
            xc1 = xT_pool.tile([P, KH, TC], bf16, tag="xc")
            for k in range(KH):
                nc.sync.dma_start(
                    out=xc1[:, k, :], in_=xT_d[k * P:(k + 1) * P, TC:2 * TC]
                )
            xT_chunks.append(xc1)
            for k in range(KQ):
                nc.sync.dma_start(out=wdn_s[:, k, :], in_=wdn_d[k * P:(k + 1) * P, :])

            for tci in range(NT):
                t0 = tci * TC
                xc = xT_chunks[tci]
                if tci + 2 < NT:
                    nxc = xT_pool.tile([P, KH, TC], bf16, tag="xc")
                    for k in range(KH):
                        nc.sync.dma_start(
                            out=nxc[:, k, :],
                            in_=xT_d[k * P:(k + 1) * P, (tci + 2) * TC:(tci + 3) * TC],
                        )
                    xT_chunks.append(nxc)
                r_c = r_pool.tile([P, TC], f32)
                nc.sync.dma_start(out=r_c, in_=rw_d[:, t0:t0 + TC].to_broadcast([P, TC]))
                hid = hid_pool.tile([P, KQ, TC], bf16)
                for qi in range(KQ):
                    gate_ps = psum_gu.tile([P, TC], f32, tag="gate")
                    up_ps = psum_gu.tile([P, TC], f32, tag="up")
                    for k in range(KH):
                        nc.tensor.matmul(
                            gate_ps,
                            wgu_s[:, 2 * qi, k, :],
                            xc[:, k, :],
                            start=(k == 0),
                            stop=(k == KH - 1),
                        )
                    for k in range(KH):
                        nc.tensor.matmul(
                            up_ps,
                            wgu_s[:, 2 * qi + 1, k, :],
                            xc[:, k, :],
                            start=(k == 0),
                            stop=(k == KH - 1),
                        )
                    tmp = tmp_pool.tile([P, TC], f32)
                    nc.scalar.activation(
                        tmp, gate_ps, mybir.ActivationFunctionType.Silu
                    )
                    nc.vector.tensor_mul(hid[:, qi, :], tmp, up_ps)

                for hi in range(NH):
                    o_ps = psum_o.tile([P, TC], f32)
                    for qi in range(KQ):
                        nc.tensor.matmul(
                            o_ps,
                            wdn_s[:, qi, hi * P:(hi + 1) * P],
                            hid[:, qi, :],
                            start=(qi == 0),
                            stop=(qi == KQ - 1),
                        )
                    ost = ost_pool.tile([P, TC], f32)
                    nc.vector.tensor_mul(ost, o_ps, r_c)
                    nc.sync.dma_start(
                        out=out_d[hi * P:(hi + 1) * P, t0:t0 + TC], in_=ost
                    )
    _split_waits(nc)
    return nc


def _get_nc():
    global _CACHED
    if _CACHED is None:
        _CACHED = _build()
    return _CACHED


def _pack_wgu(w):
    """(H, 2Q) -> (16, 128, KH, 128) bf16 slabs in first-use order:
    128-column blocks interleaved gate qi / up qi, each slab
    partition-major."""
    import ml_dtypes

    w = np.asarray(w, dtype=np.float32)
    # (KH, P, n_blk, P): k-tile, partition, column block, column
    w4 = w.reshape(KH, P, 2 * Q // P, P)
    order = [b for qi in range(KQ) for b in (qi, KQ + qi)]
    # slab s: (P, KH, P)
    return np.ascontiguousarray(
        w4.transpose(2, 1, 0, 3)[order].astype(ml_dtypes.bfloat16)
    )


def _make_in_maps(x, routing_weights, gate_up_proj, down_proj):
    import ml_dtypes

    xT = np.ascontiguousarray(
        np.asarray(x, dtype=np.float32).T.astype(ml_dtypes.bfloat16)
    )
    rw = np.asarray(routing_weights, dtype=np.float32)
    in_maps = []
    for e in range(E):
        in_maps.append({
            "xT": xT,
            "w_gu": _pack_wgu(gate_up_proj[e]),
            "w_dn": np.ascontiguousarray(
                np.asarray(down_proj[e], dtype=np.float32).astype(ml_dtypes.bfloat16)
            ),
            "rw": np.ascontiguousarray(rw[:, e].reshape(1, T)),
        })
    return in_maps


def _reduce_out(res):
    total = np.zeros((H, T), dtype=np.float32)
    for r in res.results:
        total += r["out"]
    return np.ascontiguousarray(total.T)


def kernel(x, routing_weights, gate_up_proj, down_proj):
    from concourse.bass_utils import run_bass_kernel_spmd

    nc = _get_nc()
    in_maps = _make_in_maps(x, routing_weights, gate_up_proj, down_proj)
    res = run_bass_kernel_spmd(nc, in_maps, core_ids=list(range(E)))
    return _reduce_out(res)

